# revision 1
# baseline (speedup 1.0000x reference)
"""Causal self-attention (B=2, S=2048, D=2048, H=16) on 8 trn2 NeuronCores.

Sharding: core c -> batch b = c//4, head-group hg = c%4 (4 heads of 128 dims).
Each core computes its heads' attention plus the partial output projection
(row-parallel split of W_proj); the host sums the 4 partials per batch.

All heavy matmuls run as float32r (full PE rate at free-dim >= 256); every
tensor feeding a matmul is produced as float32r end-to-end, which the BIR
verifier requires.
"""

import sys

sys.path.insert(0, "/opt/trn_rl_repo")

from contextlib import ExitStack

import numpy as np

import concourse.bass as bass
import concourse.mybir as mybir
import concourse.tile as tile
from concourse import bacc
from concourse.bass_utils import run_bass_kernel_spmd

B, S, D, H = 2, 2048, 2048, 16
HD = D // H  # 128
NH = 4  # heads per core
HG = H // NH  # head groups = 4
P = 128
KT = D // P  # 16 k-tiles over model dim
NQ = 4  # q-groups of 512
QW = S // NQ  # 512
ST = S // P  # 16 s-tiles of 128
SCALE = float(1.0 / np.sqrt(D).astype(np.float32))
MASK_NEG = -30000.0  # exp(SCALE * -30000) == 0 in fp32

F32 = mybir.dt.float32
F32R = mybir.dt.float32r


def build_bass():
    nc = bacc.Bacc("TRN2")

    xT = nc.declare_dram_parameter("xT", [D, S], F32R, isOutput=False)
    wq = nc.declare_dram_parameter("wq", [D, NH * HD], F32R, isOutput=False)
    wk = nc.declare_dram_parameter("wk", [D, NH * HD], F32R, isOutput=False)
    wv = nc.declare_dram_parameter("wv", [D, NH * HD], F32R, isOutput=False)
    wp = nc.declare_dram_parameter("wp", [NH * HD, D], F32R, isOutput=False)
    mask = nc.declare_dram_parameter("mask", [P, 3 * P], F32, isOutput=False)
    y = nc.declare_dram_parameter("y", [S, D], F32, isOutput=True)

    with tile.TileContext(nc) as tc, ExitStack() as top:
        const = top.enter_context(tc.tile_pool(name="const", bufs=1))
        dram = top.enter_context(tc.tile_pool(name="dram", bufs=1, space="DRAM"))

        # mask[:, 0:128]: triangular block mask (col >= row allowed);
        # mask[:, 128:384]: r=3 variant covering cols [256:512) of the
        # q-group (first 128 cols fully masked, last 128 triangular).
        mask_sb = const.tile([P, 3 * P], F32)
        nc.sync.dma_start(mask_sb, mask[:, :])
        ones_f32 = const.tile([P, 1], F32)
        nc.vector.memset(ones_f32, 1.0)
        ones_col = const.tile([P, 1], F32R)  # lhsT for denominator matmuls
        nc.vector.tensor_copy(ones_col, ones_f32)

        qt_scr = dram.tile([NH * HD, S], F32R)  # Q^T per head stacked
        kt_scr = dram.tile([NH * HD, S], F32R)  # K^T per head stacked
        v_scr = dram.tile([S, NH * HD], F32R)  # V natural layout

        # ---------------- Phase 1: QKV projections ----------------
        with ExitStack() as ph1:
            wpool = ph1.enter_context(tc.tile_pool(name="wpool", bufs=1))
            xpool = ph1.enter_context(tc.tile_pool(name="xpool", bufs=2))
            ppool = ph1.enter_context(tc.tile_pool(name="p1psum", bufs=4, space="PSUM"))
            bpool = ph1.enter_context(tc.tile_pool(name="p1bounce", bufs=4))

            # Load order matters: the first matmul group (n=0, h=0, wq)
            # accumulates over k-tiles in order, so k-chunked loads of
            # x-block 0 and the h=0 weight slices let PE start ~5us in
            # instead of waiting for whole tensors.
            xT_r = xT[:, :].rearrange("(k p) s -> p k s", p=P)
            wq_r = wq[:, :].rearrange("(k p) m -> p k m", p=P)
            wk_r = wk[:, :].rearrange("(k p) m -> p k m", p=P)
            xblk0 = xpool.tile([P, KT, QW], F32R, tag="xblk")
            wq_sb = wpool.tile([P, KT, NH * HD], F32R, tag="wq")
            wk_sb = wpool.tile([P, KT, NH * HD], F32R, tag="wk")
            c0s = slice(0, HD)
            for kc in range(0, KT, 4):
                ks = slice(kc, kc + 4)
                nc.sync.dma_start(xblk0[:, ks, :], xT_r[:, ks, 0:QW])
                nc.sync.dma_start(wq_sb[:, ks, c0s], wq_r[:, ks, c0s])
                nc.sync.dma_start(wk_sb[:, ks, c0s], wk_r[:, ks, c0s])
            for h in range(1, NH):
                cs = slice(h * HD, (h + 1) * HD)
                nc.sync.dma_start(wq_sb[:, :, cs], wq_r[:, :, cs])
                nc.sync.dma_start(wk_sb[:, :, cs], wk_r[:, :, cs])
            wv_sb = wpool.tile([P, KT, NH * HD], F32R, tag="wv")
            nc.sync.dma_start(wv_sb, wv[:, :].rearrange("(k p) m -> p k m", p=P))

            for n in range(NQ):
                if n == 0:
                    xblk = xblk0
                else:
                    xblk = xpool.tile([P, KT, QW], F32R, tag="xblk")
                    nc.sync.dma_start(xblk, xT_r[:, :, n * QW : (n + 1) * QW])
                for h in range(NH):
                    for w_sb, scr in ((wq_sb, qt_scr), (wk_sb, kt_scr)):
                        ps = ppool.tile([P, QW], F32, tag="ps")
                        for k in range(KT):
                            nc.tensor.matmul(
                                ps,
                                lhsT=w_sb[:, k, h * HD : (h + 1) * HD],
                                rhs=xblk[:, k, :],
                                start=(k == 0),
                                stop=(k == KT - 1),
                            )
                        bo = bpool.tile([P, QW], F32R, tag="bo")
                        nc.scalar.copy(bo, ps)
                        nc.sync.dma_start(
                            scr[h * HD : (h + 1) * HD, n * QW : (n + 1) * QW], bo
                        )
                for mi in range(4):
                    ps = ppool.tile([P, QW], F32, tag="ps")
                    for k in range(KT):
                        nc.tensor.matmul(
                            ps,
                            lhsT=xblk[:, k, mi * P : (mi + 1) * P],
                            rhs=wv_sb[:, k, :],
                            start=(k == 0),
                            stop=(k == KT - 1),
                        )
                    bo = bpool.tile([P, QW], F32R, tag="bo")
                    nc.scalar.copy(bo, ps)
                    m = n * 4 + mi
                    nc.sync.dma_start(v_scr[m * P : (m + 1) * P, :], bo)

        # ---------------- Phase 2: attention per (head, q-group) ----------------
        apool = top.enter_context(tc.tile_pool(name="apool", bufs=NH))
        wppool = top.enter_context(tc.tile_pool(name="wppool", bufs=1))
        wp_sb = wppool.tile([P, NH, D], F32R)

        a_tiles = []
        with ExitStack() as ph2:
            kvpool = ph2.enter_context(tc.tile_pool(name="kvpool", bufs=2))
            qpool = ph2.enter_context(tc.tile_pool(name="qpool", bufs=3))
            epool = ph2.enter_context(tc.tile_pool(name="epool", bufs=6))
            rpool = ph2.enter_context(tc.tile_pool(name="rpool", bufs=2))
            rbpool = ph2.enter_context(tc.tile_pool(name="rbpool", bufs=2))
            spool = ph2.enter_context(tc.tile_pool(name="spsum", bufs=4, space="PSUM"))
            upool = ph2.enter_context(tc.tile_pool(name="upsum", bufs=2, space="PSUM"))
            dpool = ph2.enter_context(tc.tile_pool(name="dpsum", bufs=2, space="PSUM"))

            v_scr_r = v_scr[:, :].rearrange("(k p) m -> p k m", p=P)
            for h in range(NH):
                if h == 1:
                    # W_proj is only needed in phase 3 — load it after the
                    # first head so it doesn't steal DMA bandwidth from the
                    # phase-1->2 transition.
                    nc.sync.dma_start(
                        wp_sb, wp[:, :].rearrange("(k p) n -> p k n", p=P)
                    )
                rs = slice(h * HD, (h + 1) * HD)
                # K^T and V stream in per q-group chunk (group qg only needs
                # k-tiles <= 4qg+3) on the scalar-engine HWDGE queue, so the
                # first scores don't wait for the full 1MB head load.
                ktsb = kvpool.tile([P, S], F32R, tag="kt")
                vsb = kvpool.tile([P, ST, HD], F32R, tag="v")
                a_h = apool.tile([P, S], F32R, tag="a", name=f"a_{h}")
                a_tiles.append(a_h)
                for qg in range(NQ):
                    qs = slice(qg * QW, (qg + 1) * QW)
                    nc.scalar.dma_start(ktsb[:, qs], kt_scr[rs, qs])
                    nc.scalar.dma_start(
                        vsb[:, 4 * qg : 4 * qg + 4, :],
                        v_scr_r[:, 4 * qg : 4 * qg + 4, rs],
                    )
                    qsb = qpool.tile([P, QW], F32R)
                    nc.sync.dma_start(qsb, qt_scr[rs, qs])
                    kmax = 4 * qg + 4
                    ups = upool.tile([P, QW], F32, tag="u")
                    dps = dpool.tile([1, QW], F32, tag="d")
                    for kt in range(kmax):
                        r = kt - 4 * qg
                        # live columns of this k-tile start at r*128; fp32r
                        # needs >=256 moving cols for full rate, so compute
                        # from c0 and mask the dead cols.
                        c0 = 0 if r < 0 else min(r * P, QW - 2 * P)
                        sps = spool.tile([P, QW], F32, tag="s")
                        nc.tensor.matmul(
                            sps[:, c0:],
                            lhsT=ktsb[:, kt * P : (kt + 1) * P],
                            rhs=qsb[:, c0:],
                            start=True,
                            stop=True,
                        )
                        if r >= 0:
                            if r == 3:
                                nc.vector.tensor_tensor(
                                    sps[:, 2 * P :], sps[:, 2 * P :],
                                    mask_sb[:, P : 3 * P],
                                    op=mybir.AluOpType.add,
                                )
                            else:
                                nc.vector.tensor_tensor(
                                    sps[:, r * P : (r + 1) * P],
                                    sps[:, r * P : (r + 1) * P],
                                    mask_sb[:, 0:P],
                                    op=mybir.AluOpType.add,
                                )
                        e = epool.tile([P, QW], F32R, tag="e")
                        nc.scalar.activation(
                            e[:, c0:], sps[:, c0:],
                            mybir.ActivationFunctionType.Exp, scale=SCALE,
                        )
                        nc.tensor.matmul(
                            ups[:, c0:],
                            lhsT=vsb[:, kt, :],
                            rhs=e[:, c0:],
                            start=(kt == 0),
                            stop=(kt == kmax - 1),
                        )
                        nc.tensor.matmul(
                            dps[:, c0:],
                            lhsT=ones_col,
                            rhs=e[:, c0:],
                            start=(kt == 0),
                            stop=(kt == kmax - 1),
                        )
                    rcp = rpool.tile([1, QW], F32)
                    nc.vector.reciprocal_approx_fast(rcp, dps)
                    rb = rbpool.tile([P, QW], F32)
                    nc.gpsimd.partition_broadcast(rb, rcp)
                    nc.vector.tensor_tensor(
                        a_h[:, qs], ups, rb, op=mybir.AluOpType.mult
                    )

        # ---------------- Phase 3: output projection ----------------
        with ExitStack() as ph3:
            ypool = ph3.enter_context(tc.tile_pool(name="ypsum", bufs=4, space="PSUM"))
            ybp = ph3.enter_context(tc.tile_pool(name="ybounce", bufs=4))
            for m in range(ST):
                for n in range(NQ):
                    yp = ypool.tile([P, QW], F32, tag="yp")
                    for k in range(NH):
                        nc.tensor.matmul(
                            yp,
                            lhsT=a_tiles[k][:, m * P : (m + 1) * P],
                            rhs=wp_sb[:, k, n * QW : (n + 1) * QW],
                            start=(k == 0),
                            stop=(k == NH - 1),
                        )
                    yb = ybp.tile([P, QW], F32, tag="yb")
                    nc.scalar.copy(yb, yp)
                    nc.sync.dma_start(
                        y[m * P : (m + 1) * P, n * QW : (n + 1) * QW], yb
                    )

    nc.finalize()
    return nc


def _build_mask():
    # [:, 0:128]   triangular block mask: allowed iff col >= row
    # [:, 128:256] all masked (r=3 variant, cols [256:384) of the q-group)
    # [:, 256:384] triangular      (r=3 variant, cols [384:512))
    # Applied pre-scale: exp(SCALE * (score + mask)).
    k = np.arange(P)[:, None]
    c = np.arange(P)[None, :]
    tri = np.where(c >= k, 0.0, MASK_NEG).astype(np.float32)
    full = np.full((P, P), MASK_NEG, dtype=np.float32)
    return np.concatenate([tri, full, tri], axis=1)


_NC_CACHE = {}


def _get_nc():
    if "nc" not in _NC_CACHE:
        _NC_CACHE["nc"] = build_bass()
    return _NC_CACHE["nc"]


def make_in_maps(x, W_qkv, W_proj):
    x = np.asarray(x, dtype=np.float32)
    W_qkv = np.asarray(W_qkv, dtype=np.float32)
    W_proj = np.asarray(W_proj, dtype=np.float32)
    Wq, Wk, Wv = W_qkv[0:D], W_qkv[D : 2 * D], W_qkv[2 * D : 3 * D]
    mask = _build_mask()
    in_maps = []
    for c in range(8):
        b, hg = c // HG, c % HG
        rows = slice(hg * NH * HD, (hg + 1) * NH * HD)
        in_maps.append(
            {
                "xT": np.ascontiguousarray(x[b].T),
                "wq": np.ascontiguousarray(Wq[rows].T),
                "wk": np.ascontiguousarray(Wk[rows].T),
                "wv": np.ascontiguousarray(Wv[rows].T),
                "wp": np.ascontiguousarray(W_proj[:, rows].T),
                "mask": mask,
            }
        )
    return in_maps


def run(x, W_qkv, W_proj, trace=False):
    nc = _get_nc()
    in_maps = make_in_maps(x, W_qkv, W_proj)
    res = run_bass_kernel_spmd(nc, in_maps, core_ids=list(range(8)), trace=trace)
    out = np.zeros((B, S, D), dtype=np.float32)
    for c in range(8):
        out[c // HG] += res.results[c]["y"]
    return out, res


def kernel(x, W_qkv, W_proj):
    out, _ = run(x, W_qkv, W_proj, trace=False)
    return out



# revision 12
# speedup vs baseline: 1.3421x; 1.3421x over previous
"""Causal self-attention (B=2, S=2048, D=2048, H=16) on 8 trn2 NeuronCores.

Sharding: core c -> batch b = c//4, head-group hg = c%4 (4 heads of 128 dims).
Each core computes its heads' attention plus the partial output projection
(row-parallel split of W_proj); the host sums the 4 partials per batch.

v3: mixed-precision, fully SBUF-resident pipeline.
 - QKV projection: seq block 0 (rows < 512) in bf16; blocks 1-3 in fp8e4
   with DoubleRow matmuls (two 128-deep k-slices contracted per pass).
   W_qkv is host-scaled by 32 so fp8 weight entries sit in the normal
   range; q/k/v come out scaled by 32.
 - Scores in bf16 (q,k at 32x -> scores at 1024x; the exp activation
   scale folds the 1/1024 back out). exp is batched per k-tile pair as
   one ACT instruction reading a [128,2,512] PSUM strip.
 - exp(scores) for kpos >= 256 is stored fp8; attention@V and the
   softmax-denominator (ones-vector) matmuls run fp8 DoubleRow for those
   pairs, bf16 for k-tiles 0/1. Short-context queries (the precision-
   critical ones) therefore never see fp8 error; long-context fp8 error
   averages out across keys.
 - Output projection in bf16 with wp host-scaled by 1/32; y written bf16
   and summed on the host in fp32.
"""

import sys

sys.path.insert(0, "/opt/trn_rl_repo")

from contextlib import ExitStack

import ml_dtypes
import numpy as np

import concourse.bass as bass
import concourse.mybir as mybir
import concourse.tile as tile
from concourse import bacc
from concourse.bass_utils import run_bass_kernel_spmd

B, S, D, H = 2, 2048, 2048, 16
HD = D // H  # 128
NH = 4  # heads per core
HG = H // NH  # head groups = 4
P = 128
KT = D // P  # 16 k-tiles over model dim
KJ = KT // 2  # 8 k-pairs for DoubleRow
NQ = 4  # seq blocks of 512
QW = S // NQ  # 512
ST = S // P  # 16 seq tiles of 128
WS = 32.0  # host-side weight scale for fp8
SCALE = float(1.0 / np.sqrt(D).astype(np.float32))
EXP_SCALE = SCALE / (WS * WS)
MASK_NEG = -1.0e9

F32 = mybir.dt.float32
BF16 = mybir.dt.bfloat16
F8 = mybir.dt.float8e4
DR = mybir.MatmulPerfMode.DoubleRow


def build_bass():
    nc = bacc.Bacc("TRN2")

    # fp8 x, pair layout: d = j*256 + i*128 + p  -> [p, j, i, s]
    x8 = nc.declare_dram_parameter("x8", [P, KJ, 2, S], F8, isOutput=False)
    # bf16 x, block 0 only: d = k*128 + p -> [p, k, s0]
    xb0 = nc.declare_dram_parameter("xb0", [P, KT, QW], BF16, isOutput=False)
    wq8 = nc.declare_dram_parameter("wq8", [P, KJ, 2, NH * HD], F8, isOutput=False)
    wk8 = nc.declare_dram_parameter("wk8", [P, KJ, 2, NH * HD], F8, isOutput=False)
    wv8 = nc.declare_dram_parameter("wv8", [P, KJ, 2, NH * HD], F8, isOutput=False)
    wqb = nc.declare_dram_parameter("wqb", [P, KT, NH * HD], BF16, isOutput=False)
    wkb = nc.declare_dram_parameter("wkb", [P, KT, NH * HD], BF16, isOutput=False)
    wvb = nc.declare_dram_parameter("wvb", [P, KT, NH * HD], BF16, isOutput=False)
    wp = nc.declare_dram_parameter("wp", [P, NH, D], BF16, isOutput=False)
    # mask strip: [:, 0:128] triangular (col>=row allowed), [:, 128:384]
    # dead(128) | triangular(128)
    mask = nc.declare_dram_parameter("mask", [P, 3 * P], F32, isOutput=False)
    y = nc.declare_dram_parameter("y", [S, D], BF16, isOutput=True)

    with tile.TileContext(nc) as tc, ExitStack() as top:
        const = top.enter_context(tc.tile_pool(name="const", bufs=1))
        persist = top.enter_context(tc.tile_pool(name="persist", bufs=1))

        mask_sb = const.tile([P, 3 * P], F32)
        nc.sync.dma_start(mask_sb, mask[:, :])
        ones_b = const.tile([P, P], BF16)
        nc.vector.memset(ones_b, 1.0)
        ones8 = const.tile([P, 2, P], F8)
        nc.vector.memset(ones8, 1.0)

        qh = persist.tile([P, NH, S], BF16)  # Q^T per head (32x scale)
        kh = persist.tile([P, NH, S], BF16)  # K^T per head (32x scale)
        v_lo = persist.tile([P, 2, NH * HD], BF16)  # v rows < 256 (32x)
        v_hi = persist.tile([P, ST - 2, NH * HD], F8)  # v rows >= 256 (32x)
        a_sb = persist.tile([P, NH, S], BF16)  # attention out (32x)
        wp_sb = persist.tile([P, NH, D], BF16)  # W_proj slice (1/32 scale)

        ph1a = ExitStack()  # bf16 inputs: freed after the early bf16 matmuls
        ph1 = ExitStack()   # fp8 inputs: freed after the last DR matmul
        ph1_sb = ph1.enter_context(tc.tile_pool(name="ph1sb", bufs=1, side="right"))
        ph1_ps = ph1.enter_context(
            tc.tile_pool(name="ph1ps", bufs=2, space="PSUM", side="right")
        )
        ph1a_sb = ph1a.enter_context(
            tc.tile_pool(name="ph1asb", bufs=1, side="right")
        )

        xb0_sb = ph1a_sb.tile([P, KT, QW], BF16)
        wqb_sb = ph1a_sb.tile([P, KT, NH * HD], BF16)
        wkb_sb = ph1a_sb.tile([P, KT, NH * HD], BF16)
        wvb_sb = ph1a_sb.tile([P, KT, NH * HD], BF16)
        x8_sb = ph1_sb.tile([P, KJ, 2, S], F8)
        wq8_sb = ph1_sb.tile([P, KJ, 2, NH * HD], F8)
        wk8_sb = ph1_sb.tile([P, KJ, 2, NH * HD], F8)
        wv8_sb = ph1_sb.tile([P, KJ, 2, NH * HD], F8)

        # --- DMA schedule ---
        # sync queue: bf16 x block + bf16 q/k weights, chunked by k-pairs so
        # the first matmuls start ~2us in.
        for kc in range(0, KT, 2):
            ks = slice(kc, kc + 2)
            nc.sync.dma_start(xb0_sb[:, ks, :], xb0[:, ks, :])
            nc.sync.dma_start(wqb_sb[:, ks, :], wqb[:, ks, :])
            nc.sync.dma_start(wkb_sb[:, ks, :], wkb[:, ks, :])
        # scalar queue: fp8 tensors + bf16 v weights (needed later).
        nc.scalar.dma_start(wvb_sb, wvb[:, :, :])
        for jc in range(0, KJ, 2):
            js = slice(jc, jc + 2)
            nc.scalar.dma_start(x8_sb[:, js, :, :], x8[:, js, :, :])
        nc.scalar.dma_start(wq8_sb, wq8[:, :, :, :])
        nc.scalar.dma_start(wk8_sb, wk8[:, :, :, :])
        nc.scalar.dma_start(wv8_sb, wv8[:, :, :, :])
        # gpsimd queue: W_proj (needed only in phase 3).
        nc.gpsimd.dma_start(wp_sb, wp[:, :, :])

        def qk_block0(h):
            """bf16 q,k for head h, seq rows 0:512."""
            cs = slice(h * HD, (h + 1) * HD)
            for w_sb, out in ((wqb_sb, qh), (wkb_sb, kh)):
                ps = ph1_ps.tile([P, QW], F32, tag="ps")
                for k in range(KT):
                    nc.tensor.matmul(
                        ps,
                        lhsT=w_sb[:, k, cs],
                        rhs=xb0_sb[:, k, :],
                        start=(k == 0),
                        stop=(k == KT - 1),
                    )
                nc.vector.tensor_copy(out[:, h, 0:QW], ps)

        def qk_dr(h, n):
            """fp8 DoubleRow q,k for head h, seq block n (1..3)."""
            cs = slice(h * HD, (h + 1) * HD)
            ss = slice(n * QW, (n + 1) * QW)
            for w_sb, out in ((wq8_sb, qh), (wk8_sb, kh)):
                ps = ph1_ps.tile([P, QW], F32, tag="ps")
                for j in range(KJ):
                    nc.tensor.matmul(
                        ps,
                        lhsT=w_sb[:, j, :, cs],
                        rhs=x8_sb[:, j, :, ss],
                        start=(j == 0),
                        stop=(j == KJ - 1),
                        perf_mode=DR,
                    )
                nc.vector.tensor_copy(out[:, h, ss], ps)

        def v_block(m):
            """v rows m*128:(m+1)*128 (all 4 heads)."""
            ms = slice(m * P, (m + 1) * P)
            ps = ph1_ps.tile([P, QW], F32, tag="ps")
            if m < 2:
                for k in range(KT):
                    nc.tensor.matmul(
                        ps,
                        lhsT=xb0_sb[:, k, ms],
                        rhs=wvb_sb[:, k, :],
                        start=(k == 0),
                        stop=(k == KT - 1),
                    )
                nc.vector.tensor_copy(v_lo[:, m, :], ps)
            else:
                for j in range(KJ):
                    nc.tensor.matmul(
                        ps,
                        lhsT=x8_sb[:, j, :, ms],
                        rhs=wv8_sb[:, j, :, :],
                        start=(j == 0),
                        stop=(j == KJ - 1),
                        perf_mode=DR,
                    )
                nc.vector.tensor_copy(v_hi[:, m - 2, :], ps)

        # ---- phase 2 machinery ----
        ph2 = ExitStack()
        ph3 = ExitStack()

        def attn_head(h, pools):
            for qg in range(NQ):
                _attn_one_qg(nc, h, qg, mask_sb, ones_b, ones8, qh, kh, v_lo,
                             v_hi, a_sb, pools)

        def proj_mtile(m, ypool, ybpool):
            """output y rows m*128:(m+1)*128 (all 2048 cols)."""
            for n in range(NQ):
                yp = ypool.tile([P, QW], F32, tag="yp")
                for hh in range(NH):
                    nc.tensor.matmul(
                        yp,
                        lhsT=a_sb[:, hh, m * P : (m + 1) * P],
                        rhs=wp_sb[:, hh, n * QW : (n + 1) * QW],
                        start=(hh == 0),
                        stop=(hh == NH - 1),
                    )
                yb = ybpool.tile([P, QW], BF16, tag="yb")
                nc.scalar.copy(yb, yp)
                nc.sync.dma_start(
                    y[m * P : (m + 1) * P, n * QW : (n + 1) * QW], yb
                )

        # ---- emission schedule ----
        for h in range(NH):
            qk_block0(h)
        for m in range(2):
            v_block(m)
        ph1a.close()
        for m in range(2, ST):
            v_block(m)
        qk_dr(0, 1)
        qk_dr(0, 2)
        qk_dr(0, 3)
        for n in range(1, NQ):
            qk_dr(1, n)

        spool = ph2.enter_context(tc.tile_pool(name="spsum", bufs=2, space="PSUM"))
        upool = ph2.enter_context(tc.tile_pool(name="upsum", bufs=1, space="PSUM"))
        dpool = ph2.enter_context(tc.tile_pool(name="dpsum", bufs=1, space="PSUM"))
        elpool = ph2.enter_context(tc.tile_pool(name="elpool", bufs=2))
        ehpool = ph2.enter_context(tc.tile_pool(name="ehpool", bufs=3))
        rpool = ph2.enter_context(tc.tile_pool(name="rpool", bufs=2))
        ph2_pools = (spool, elpool, ehpool, upool, dpool, rpool)

        attn_head(0, ph2_pools)
        for n in range(1, NQ):
            qk_dr(2, n)
        attn_head(1, ph2_pools)
        for n in range(1, NQ):
            qk_dr(3, n)
        ph1.close()

        ypool = ph3.enter_context(tc.tile_pool(name="ypsum", bufs=2, space="PSUM"))
        ybpool = ph3.enter_context(tc.tile_pool(name="ybounce", bufs=4))
        attn_head(2, ph2_pools)
        # head 3 interleaved with the output projection: after h3 finishes
        # q-group qg, rows 4qg..4qg+3 of y are fully determined.
        for qg in range(NQ):
            _attn_one_qg(nc, 3, qg, mask_sb, ones_b, ones8, qh, kh, v_lo,
                         v_hi, a_sb, ph2_pools)
            for mi in range(4):
                proj_mtile(4 * qg + mi, ypool, ybpool)
        ph3.close()
        ph2.close()

    nc.finalize()
    return nc


def _attn_one_qg(nc, h, qg, mask_sb, ones_b, ones8, qh, kh, v_lo, v_hi,
                 a_sb, pools):
    spool, elpool, ehpool, upool, dpool, rpool = pools
    rs = slice(h * HD, (h + 1) * HD)
    kmax = 4 * qg + 4
    npairs = kmax // 2
    ups = upool.tile([P, QW], F32, tag="u")
    dps = dpool.tile([P, QW], F32, tag="d")
    for j in range(npairs):
        kt0 = 2 * j
        is_d0 = j == 2 * qg
        is_d1 = j == 2 * qg + 1
        c0 = 2 * P if is_d1 else 0
        bf = j == 0
        sps = spool.tile([P, 2, QW], F32, tag="s")
        for i in range(2):
            kt = kt0 + i
            nc.tensor.matmul(
                sps[:, i, c0:],
                lhsT=kh[:, h, kt * P : (kt + 1) * P],
                rhs=qh[:, h, qg * QW + c0 : (qg + 1) * QW],
                start=True,
                stop=True,
            )
        if is_d0:
            nc.vector.tensor_tensor(
                sps[:, 0, 0:P], sps[:, 0, 0:P], mask_sb[:, 0:P],
                op=mybir.AluOpType.add,
            )
            nc.vector.tensor_tensor(
                sps[:, 1, 0 : 2 * P], sps[:, 1, 0 : 2 * P],
                mask_sb[:, P : 3 * P],
                op=mybir.AluOpType.add,
            )
        elif is_d1:
            nc.vector.tensor_tensor(
                sps[:, 0, 2 * P : 3 * P], sps[:, 0, 2 * P : 3 * P],
                mask_sb[:, 0:P],
                op=mybir.AluOpType.add,
            )
            nc.vector.tensor_tensor(
                sps[:, 1, 2 * P :], sps[:, 1, 2 * P :],
                mask_sb[:, P : 3 * P],
                op=mybir.AluOpType.add,
            )
        if bf:
            e = elpool.tile([P, 2, QW], BF16, tag="el")
        else:
            e = ehpool.tile([P, 2, QW], F8, tag="eh")
        nc.scalar.activation(
            e[:, :, c0:], sps[:, :, c0:],
            mybir.ActivationFunctionType.Exp, scale=EXP_SCALE,
        )
        last = j == npairs - 1
        if bf:
            for i in range(2):
                nc.tensor.matmul(
                    ups, lhsT=v_lo[:, i, rs], rhs=e[:, i, :],
                    start=(i == 0), stop=False,
                )
                nc.tensor.matmul(
                    dps, lhsT=ones_b, rhs=e[:, i, :],
                    start=(i == 0), stop=False,
                )
        else:
            nc.tensor.matmul(
                ups[:, c0:], lhsT=v_hi[:, kt0 - 2 : kt0, rs],
                rhs=e[:, :, c0:], start=False, stop=last, perf_mode=DR,
            )
            nc.tensor.matmul(
                dps[:, c0:], lhsT=ones8, rhs=e[:, :, c0:],
                start=False, stop=last, perf_mode=DR,
            )
    rcp = rpool.tile([P, QW], F32, tag="r")
    nc.vector.reciprocal_approx_fast(rcp, dps)
    nc.vector.tensor_tensor(
        a_sb[:, h, qg * QW : (qg + 1) * QW], ups, rcp,
        op=mybir.AluOpType.mult,
    )


def _build_mask():
    k = np.arange(P)[:, None]
    c = np.arange(P)[None, :]
    tri = np.where(c >= k, 0.0, MASK_NEG).astype(np.float32)
    dead = np.full((P, P), MASK_NEG, dtype=np.float32)
    return np.concatenate([tri, dead, tri], axis=1)


_NC_CACHE = {}


def _get_nc():
    if "nc" not in _NC_CACHE:
        _NC_CACHE["nc"] = build_bass()
    return _NC_CACHE["nc"]


def make_in_maps(x, W_qkv, W_proj):
    x = np.asarray(x, dtype=np.float32)
    W_qkv = np.asarray(W_qkv, dtype=np.float32)
    W_proj = np.asarray(W_proj, dtype=np.float32)
    Wq, Wk, Wv = W_qkv[0:D], W_qkv[D : 2 * D], W_qkv[2 * D : 3 * D]
    mask = _build_mask()

    def pair8(a):  # [D, M] fp32 -> [P, KJ, 2, M] fp8 (d = j*256+i*128+p)
        a8 = (a * WS).astype(ml_dtypes.float8_e4m3)
        return np.ascontiguousarray(
            a8.reshape(KJ, 2, P, a.shape[1]).transpose(2, 0, 1, 3)
        )

    def kt16(a):  # [D, M] fp32 -> [P, KT, M] bf16 (d = k*128+p), scaled
        ab = (a * WS).astype(ml_dtypes.bfloat16)
        return np.ascontiguousarray(ab.reshape(KT, P, -1).transpose(1, 0, 2))

    # per-batch tensors (shared by 4 cores each)
    xT = [np.ascontiguousarray(x[b].T) for b in range(B)]  # [D, S]
    x8_b = [
        np.ascontiguousarray(
            xT[b].astype(ml_dtypes.float8_e4m3).reshape(KJ, 2, P, S).transpose(2, 0, 1, 3)
        )
        for b in range(B)
    ]
    xb0_b = [
        np.ascontiguousarray(
            xT[b][:, 0:QW].astype(ml_dtypes.bfloat16).reshape(KT, P, QW).transpose(1, 0, 2)
        )
        for b in range(B)
    ]
    # per head-group weight slices (shared across batches)
    w_slices = []
    for hg in range(HG):
        rows = slice(hg * NH * HD, (hg + 1) * NH * HD)
        wqT = np.ascontiguousarray(Wq[rows].T)  # [D, 512]
        wkT = np.ascontiguousarray(Wk[rows].T)
        wvT = np.ascontiguousarray(Wv[rows].T)
        wpT = np.ascontiguousarray(W_proj[:, rows].T)  # [512, D]
        w_slices.append(
            {
                "wq8": pair8(wqT),
                "wk8": pair8(wkT),
                "wv8": pair8(wvT),
                "wqb": kt16(wqT),
                "wkb": kt16(wkT),
                "wvb": kt16(wvT),
                "wp": np.ascontiguousarray(
                    (wpT / WS).astype(ml_dtypes.bfloat16).reshape(NH, P, D).transpose(1, 0, 2)
                ),
            }
        )

    in_maps = []
    for c in range(8):
        b, hg = c // HG, c % HG
        m = {"x8": x8_b[b], "xb0": xb0_b[b], "mask": mask}
        m.update(w_slices[hg])
        in_maps.append(m)
    return in_maps


def run(x, W_qkv, W_proj, trace=False):
    nc = _get_nc()
    in_maps = make_in_maps(x, W_qkv, W_proj)
    res = run_bass_kernel_spmd(nc, in_maps, core_ids=list(range(8)), trace=trace)
    out = np.zeros((B, S, D), dtype=np.float32)
    for c in range(8):
        out[c // HG] += res.results[c]["y"].astype(np.float32)
    return out, res


def kernel(x, W_qkv, W_proj):
    out, _ = run(x, W_qkv, W_proj, trace=False)
    return out


# revision 16
# speedup vs baseline: 1.4085x; 1.0495x over previous
"""Causal self-attention (B=2, S=2048, D=2048, H=16) on 8 trn2 NeuronCores.

Sharding: core c -> batch b = c//4, head-group hg = c%4 (4 heads of 128 dims).
Each core computes its heads' attention plus the partial output projection
(row-parallel split of W_proj); the host sums the 4 partials per batch.

Mixed-precision, fully SBUF-resident pipeline:
 - QKV projection: seq rows < 256 in bf16; the rest in fp8e4 with DoubleRow
   matmuls (two 128-deep k-slices contracted per instruction). W_qkv is
   host-scaled by 32 so fp8 weight entries sit in the normal range; q/k/v
   come out scaled by 32 (wp is host-scaled by 1/32 to compensate).
 - Scores in bf16 (q,k at 32x -> scores at 1024x; the exp activation scale
   folds the 1/1024 back out). exp is batched per k-tile pair as one ACT
   instruction reading a [128,2,512] PSUM strip.
 - attention@V and the softmax-denominator matmuls run fp8 DoubleRow
   everywhere except (q-group 0, k-tiles 0/1), which stays bf16: short-
   context queries are the precision-critical ones; long-context fp8
   error averages out across keys. The denominator uses a ones *matrix*
   as the stationary operand, so it lands PSUM-replicated across all 128
   partitions and needs no broadcast before the reciprocal multiply.
 - Output projection bf16, interleaved per q-group with head 3's
   attention so PE fills the exp-latency gaps; y written bf16 and summed
   on the host in fp32.
"""

import sys

sys.path.insert(0, "/opt/trn_rl_repo")

from contextlib import ExitStack

import ml_dtypes
import numpy as np

import concourse.bass as bass
import concourse.mybir as mybir
import concourse.tile as tile
from concourse import bacc
from concourse.bass_utils import run_bass_kernel_spmd

B, S, D, H = 2, 2048, 2048, 16
HD = D // H  # 128
NH = 4  # heads per core
HG = H // NH  # head groups = 4
P = 128
KT = D // P  # 16 k-tiles over model dim
KJ = KT // 2  # 8 k-pairs for DoubleRow
NQ = 4  # seq blocks of 512
QW = S // NQ  # 512
ST = S // P  # 16 seq tiles of 128
BW = 256  # bf16 window: seq rows [0, BW) use bf16 QKV / e / v
WS = 32.0  # host-side weight scale for fp8
SCALE = float(1.0 / np.sqrt(D).astype(np.float32))
EXP_SCALE = SCALE / (WS * WS)
MASK_NEG = -1.0e9

F32 = mybir.dt.float32
BF16 = mybir.dt.bfloat16
F8 = mybir.dt.float8e4
DR = mybir.MatmulPerfMode.DoubleRow


def build_bass():
    nc = bacc.Bacc("TRN2")

    # fp8 x, pair layout: d = j*256 + i*128 + p  -> [p, j, i, s]
    x8 = nc.declare_dram_parameter("x8", [P, KJ, 2, S], F8, isOutput=False)
    # bf16 x, seq rows [0, 256): d = k*128 + p -> [p, k, s]
    xb0 = nc.declare_dram_parameter("xb0", [P, KT, BW], BF16, isOutput=False)
    wq8 = nc.declare_dram_parameter("wq8", [P, KJ, 2, NH * HD], F8, isOutput=False)
    wk8 = nc.declare_dram_parameter("wk8", [P, KJ, 2, NH * HD], F8, isOutput=False)
    wv8 = nc.declare_dram_parameter("wv8", [P, KJ, 2, NH * HD], F8, isOutput=False)
    wqb = nc.declare_dram_parameter("wqb", [P, KT, NH * HD], BF16, isOutput=False)
    wkb = nc.declare_dram_parameter("wkb", [P, KT, NH * HD], BF16, isOutput=False)
    wvb = nc.declare_dram_parameter("wvb", [P, KT, NH * HD], BF16, isOutput=False)
    wp = nc.declare_dram_parameter("wp", [P, NH, D], BF16, isOutput=False)
    # mask strip: [:, 0:128] triangular (col>=row allowed), [:, 128:384]
    # dead(128) | triangular(128)
    mask = nc.declare_dram_parameter("mask", [P, 3 * P], F32, isOutput=False)
    y = nc.declare_dram_parameter("y", [S, D], BF16, isOutput=True)

    with tile.TileContext(nc) as tc, ExitStack() as top:
        const = top.enter_context(tc.tile_pool(name="const", bufs=1))
        persist = top.enter_context(tc.tile_pool(name="persist", bufs=1))

        mask_sb = const.tile([P, 3 * P], F32)
        nc.sync.dma_start(mask_sb, mask[:, :])
        ones_b = const.tile([P, P], BF16)
        nc.vector.memset(ones_b, 1.0)
        ones8 = const.tile([P, 2, P], F8)
        nc.vector.memset(ones8, 1.0)

        qh = persist.tile([P, NH, S], BF16)  # Q^T per head (32x scale)
        kh = persist.tile([P, NH, S], BF16)  # K^T per head (32x scale)
        v_lo = persist.tile([P, 2, NH * HD], BF16)  # v rows < 256 (32x)
        v8 = persist.tile([P, ST, NH * HD], F8)  # all v rows, fp8 (32x)
        a_sb = persist.tile([P, NH, S], BF16)  # attention out (32x)
        wp_sb = persist.tile([P, NH, D], BF16)  # W_proj slice (1/32 scale)

        ph1a = ExitStack()  # bf16 inputs: freed after the early bf16 matmuls
        ph1 = ExitStack()   # fp8 inputs: freed after the last DR matmul
        ph1_sb = ph1.enter_context(tc.tile_pool(name="ph1sb", bufs=1, side="right"))
        ph1_ps = ph1.enter_context(
            tc.tile_pool(name="ph1ps", bufs=2, space="PSUM", side="right")
        )
        ph1a_sb = ph1a.enter_context(
            tc.tile_pool(name="ph1asb", bufs=1, side="right")
        )

        xb0_sb = ph1a_sb.tile([P, KT, BW], BF16)
        wqb_sb = ph1a_sb.tile([P, KT, NH * HD], BF16)
        wkb_sb = ph1a_sb.tile([P, KT, NH * HD], BF16)
        wvb_sb = ph1a_sb.tile([P, KT, NH * HD], BF16)
        x8_sb = ph1_sb.tile([P, KJ, 2, S], F8)
        wq8_sb = ph1_sb.tile([P, KJ, 2, NH * HD], F8)
        wk8_sb = ph1_sb.tile([P, KJ, 2, NH * HD], F8)
        wv8_sb = ph1_sb.tile([P, KJ, 2, NH * HD], F8)

        # --- DMA schedule: three queues so no consumer starves ---
        # sync: xb0 + bf16 q weights chunk-interleaved, then fp8 q/k weights
        for kc in range(0, KT, 2):
            ks = slice(kc, kc + 2)
            nc.sync.dma_start(xb0_sb[:, ks, :], xb0[:, ks, :])
            nc.sync.dma_start(wqb_sb[:, ks, :], wqb[:, ks, :])
        nc.sync.dma_start(wq8_sb, wq8[:, :, :, :])
        nc.sync.dma_start(wk8_sb, wk8[:, :, :, :])
        # gpsimd: bf16 k weights chunked, then bf16 v weights, then W_proj
        for kc in range(0, KT, 2):
            nc.gpsimd.dma_start(
                wkb_sb[:, kc : kc + 2, :], wkb[:, kc : kc + 2, :]
            )
        nc.gpsimd.dma_start(wvb_sb, wvb[:, :, :])
        nc.gpsimd.dma_start(wp_sb, wp[:, :, :])
        # scalar: fp8 v weights first (v DR needs them early), then fp8 x
        nc.scalar.dma_start(wv8_sb, wv8[:, :, :, :])
        for jc in range(0, KJ, 2):
            js = slice(jc, jc + 2)
            nc.scalar.dma_start(x8_sb[:, js, :, :], x8[:, js, :, :])

        def qk_bf(h):
            """bf16 q,k for head h, seq rows 0:256."""
            cs = slice(h * HD, (h + 1) * HD)
            for w_sb, out in ((wqb_sb, qh), (wkb_sb, kh)):
                ps = ph1_ps.tile([P, QW], F32, tag="ps")
                for k in range(KT):
                    nc.tensor.matmul(
                        ps[:, 0:BW],
                        lhsT=w_sb[:, k, cs],
                        rhs=xb0_sb[:, k, :],
                        start=(k == 0),
                        stop=(k == KT - 1),
                    )
                nc.vector.tensor_copy(out[:, h, 0:BW], ps[:, 0:BW])

        def qk_dr(h, lo, hi):
            """fp8 DoubleRow q,k for head h, seq cols [lo, hi)."""
            cs = slice(h * HD, (h + 1) * HD)
            for w_sb, out in ((wq8_sb, qh), (wk8_sb, kh)):
                ps = ph1_ps.tile([P, QW], F32, tag="ps")
                for j in range(KJ):
                    nc.tensor.matmul(
                        ps[:, 0 : hi - lo],
                        lhsT=w_sb[:, j, :, cs],
                        rhs=x8_sb[:, j, :, lo:hi],
                        start=(j == 0),
                        stop=(j == KJ - 1),
                        perf_mode=DR,
                    )
                nc.vector.tensor_copy(out[:, h, lo:hi], ps[:, 0 : hi - lo])

        def v_block(m):
            """v rows m*128:(m+1)*128 (all 4 heads)."""
            ms = slice(m * P, (m + 1) * P)
            ps = ph1_ps.tile([P, QW], F32, tag="ps")
            if m < 2:
                for k in range(KT):
                    nc.tensor.matmul(
                        ps,
                        lhsT=xb0_sb[:, k, ms],
                        rhs=wvb_sb[:, k, :],
                        start=(k == 0),
                        stop=(k == KT - 1),
                    )
                nc.vector.tensor_copy(v_lo[:, m, :], ps)
                nc.vector.tensor_copy(v8[:, m, :], ps)
            else:
                for j in range(KJ):
                    nc.tensor.matmul(
                        ps,
                        lhsT=x8_sb[:, j, :, ms],
                        rhs=wv8_sb[:, j, :, :],
                        start=(j == 0),
                        stop=(j == KJ - 1),
                        perf_mode=DR,
                    )
                nc.vector.tensor_copy(v8[:, m, :], ps)

        def attn_one_qg(h, qg, pools):
            spool, elpool, ehpool, upool, dpool, rpool = pools
            rs = slice(h * HD, (h + 1) * HD)
            kmax = 4 * qg + 4
            npairs = kmax // 2
            ups = upool.tile([P, QW], F32, tag="u")
            dps = dpool.tile([P, QW], F32, tag="d")
            for j in range(npairs):
                kt0 = 2 * j
                is_d0 = j == 2 * qg
                is_d1 = j == 2 * qg + 1
                c0 = 2 * P if is_d1 else 0
                bf = j == 0 and qg == 0
                sps = spool.tile([P, 2, QW], F32, tag="s")
                for i in range(2):
                    kt = kt0 + i
                    # live window of this tile's scores
                    if is_d0:
                        sc0 = i * P  # r0: full, r1: cols >= 128
                    elif is_d1:
                        sc0 = 2 * P + i * P  # r2: >= 256, r3: >= 384
                    else:
                        sc0 = 0
                    nc.tensor.matmul(
                        sps[:, i, sc0:],
                        lhsT=kh[:, h, kt * P : (kt + 1) * P],
                        rhs=qh[:, h, qg * QW + sc0 : (qg + 1) * QW],
                        start=True,
                        stop=True,
                    )
                if is_d0:
                    nc.vector.tensor_tensor(
                        sps[:, 0, 0:P], sps[:, 0, 0:P], mask_sb[:, 0:P],
                        op=mybir.AluOpType.add,
                    )
                    nc.vector.tensor_tensor(
                        sps[:, 1, 0 : 2 * P], sps[:, 1, 0 : 2 * P],
                        mask_sb[:, P : 3 * P],
                        op=mybir.AluOpType.add,
                    )
                elif is_d1:
                    nc.vector.tensor_tensor(
                        sps[:, 0, 2 * P : 3 * P], sps[:, 0, 2 * P : 3 * P],
                        mask_sb[:, 0:P],
                        op=mybir.AluOpType.add,
                    )
                    nc.vector.tensor_tensor(
                        sps[:, 1, 2 * P :], sps[:, 1, 2 * P :],
                        mask_sb[:, P : 3 * P],
                        op=mybir.AluOpType.add,
                    )
                if bf:
                    e = elpool.tile([P, 2, QW], BF16, tag="el")
                else:
                    e = ehpool.tile([P, 2, QW], F8, tag="eh")
                nc.scalar.activation(
                    e[:, :, c0:], sps[:, :, c0:],
                    mybir.ActivationFunctionType.Exp, scale=EXP_SCALE,
                )
                last = j == npairs - 1
                if bf:
                    for i in range(2):
                        nc.tensor.matmul(
                            ups,
                            lhsT=v_lo[:, i, rs],
                            rhs=e[:, i, :],
                            start=(i == 0),
                            stop=False,
                        )
                        nc.tensor.matmul(
                            dps,
                            lhsT=ones_b,
                            rhs=e[:, i, :],
                            start=(i == 0),
                            stop=False,
                        )
                else:
                    first = j == 0
                    nc.tensor.matmul(
                        ups[:, c0:],
                        lhsT=v8[:, kt0 : kt0 + 2, rs],
                        rhs=e[:, :, c0:],
                        start=first,
                        stop=last,
                        perf_mode=DR,
                    )
                    nc.tensor.matmul(
                        dps[:, c0:],
                        lhsT=ones8,
                        rhs=e[:, :, c0:],
                        start=first,
                        stop=last,
                        perf_mode=DR,
                    )
            rcp = rpool.tile([P, QW], F32, tag="r")
            nc.vector.reciprocal_approx_fast(rcp, dps)
            nc.vector.tensor_tensor(
                a_sb[:, h, qg * QW : (qg + 1) * QW], ups, rcp,
                op=mybir.AluOpType.mult,
            )

        def proj_mtile(m, ypool, ybpool):
            """output y rows m*128:(m+1)*128 (all 2048 cols)."""
            qdma = [nc.sync, nc.gpsimd, nc.sync, nc.gpsimd]
            for n in range(NQ):
                yp = ypool.tile([P, QW], F32, tag="yp")
                for hh in range(NH):
                    nc.tensor.matmul(
                        yp,
                        lhsT=a_sb[:, hh, m * P : (m + 1) * P],
                        rhs=wp_sb[:, hh, n * QW : (n + 1) * QW],
                        start=(hh == 0),
                        stop=(hh == NH - 1),
                    )
                yb = ybpool.tile([P, QW], BF16, tag="yb")
                nc.scalar.copy(yb, yp)
                qdma[n].dma_start(
                    y[m * P : (m + 1) * P, n * QW : (n + 1) * QW], yb
                )

        # ---- emission schedule ----
        for h in range(NH):
            qk_bf(h)
        for m in range(2):
            v_block(m)
        ph1a.close()
        for m in range(2, ST):
            v_block(m)
        for h in range(NH):
            qk_dr(h, BW, QW)
        for n in range(1, NQ):
            for h in range(NH):
                qk_dr(h, n * QW, (n + 1) * QW)
        ph1.close()

        ph2 = ExitStack()
        ph3 = ExitStack()
        spool = ph2.enter_context(tc.tile_pool(name="spsum", bufs=2, space="PSUM"))
        upool = ph2.enter_context(tc.tile_pool(name="upsum", bufs=1, space="PSUM"))
        dpool = ph2.enter_context(tc.tile_pool(name="dpsum", bufs=1, space="PSUM"))
        elpool = ph2.enter_context(tc.tile_pool(name="elpool", bufs=2))
        ehpool = ph2.enter_context(tc.tile_pool(name="ehpool", bufs=3))
        rpool = ph2.enter_context(tc.tile_pool(name="rpool", bufs=2))
        ph2_pools = (spool, elpool, ehpool, upool, dpool, rpool)
        ypool = ph3.enter_context(tc.tile_pool(name="ypsum", bufs=2, space="PSUM"))
        ybpool = ph3.enter_context(tc.tile_pool(name="ybounce", bufs=4))

        # q-group major: after all heads finish q-group qg, y rows
        # 4qg..4qg+3 are fully determined -> the projection matmuls fill
        # the exp-latency gaps of the next q-group.
        for qg in range(NQ):
            for h in range(NH):
                attn_one_qg(h, qg, ph2_pools)
            for mi in range(4):
                proj_mtile(4 * qg + mi, ypool, ybpool)
        ph3.close()
        ph2.close()

    nc.finalize()
    return nc


def _build_mask():
    k = np.arange(P)[:, None]
    c = np.arange(P)[None, :]
    tri = np.where(c >= k, 0.0, MASK_NEG).astype(np.float32)
    dead = np.full((P, P), MASK_NEG, dtype=np.float32)
    return np.concatenate([tri, dead, tri], axis=1)


_NC_CACHE = {}


def _get_nc():
    if "nc" not in _NC_CACHE:
        _NC_CACHE["nc"] = build_bass()
    return _NC_CACHE["nc"]


def make_in_maps(x, W_qkv, W_proj):
    x = np.asarray(x, dtype=np.float32)
    W_qkv = np.asarray(W_qkv, dtype=np.float32)
    W_proj = np.asarray(W_proj, dtype=np.float32)
    Wq, Wk, Wv = W_qkv[0:D], W_qkv[D : 2 * D], W_qkv[2 * D : 3 * D]
    mask = _build_mask()

    def pair8(a):  # [D, M] fp32 -> [P, KJ, 2, M] fp8 (d = j*256+i*128+p)
        a8 = (a * WS).astype(ml_dtypes.float8_e4m3)
        return np.ascontiguousarray(
            a8.reshape(KJ, 2, P, a.shape[1]).transpose(2, 0, 1, 3)
        )

    def kt16(a):  # [D, M] fp32 -> [P, KT, M] bf16 (d = k*128+p), scaled
        ab = (a * WS).astype(ml_dtypes.bfloat16)
        return np.ascontiguousarray(ab.reshape(KT, P, -1).transpose(1, 0, 2))

    # per-batch tensors (shared by 4 cores each)
    xT = [np.ascontiguousarray(x[b].T) for b in range(B)]  # [D, S]
    x8_b = [
        np.ascontiguousarray(
            xT[b].astype(ml_dtypes.float8_e4m3).reshape(KJ, 2, P, S).transpose(2, 0, 1, 3)
        )
        for b in range(B)
    ]
    xb0_b = [
        np.ascontiguousarray(
            xT[b][:, 0:BW].astype(ml_dtypes.bfloat16).reshape(KT, P, BW).transpose(1, 0, 2)
        )
        for b in range(B)
    ]
    # per head-group weight slices (shared across batches)
    w_slices = []
    for hg in range(HG):
        rows = slice(hg * NH * HD, (hg + 1) * NH * HD)
        wqT = np.ascontiguousarray(Wq[rows].T)  # [D, 512]
        wkT = np.ascontiguousarray(Wk[rows].T)
        wvT = np.ascontiguousarray(Wv[rows].T)
        wpT = np.ascontiguousarray(W_proj[:, rows].T)  # [512, D]
        w_slices.append(
            {
                "wq8": pair8(wqT),
                "wk8": pair8(wkT),
                "wv8": pair8(wvT),
                "wqb": kt16(wqT),
                "wkb": kt16(wkT),
                "wvb": kt16(wvT),
                "wp": np.ascontiguousarray(
                    (wpT / WS).astype(ml_dtypes.bfloat16).reshape(NH, P, D).transpose(1, 0, 2)
                ),
            }
        )

    in_maps = []
    for c in range(8):
        b, hg = c // HG, c % HG
        m = {"x8": x8_b[b], "xb0": xb0_b[b], "mask": mask}
        m.update(w_slices[hg])
        in_maps.append(m)
    return in_maps


def run(x, W_qkv, W_proj, trace=False):
    nc = _get_nc()
    in_maps = make_in_maps(x, W_qkv, W_proj)
    res = run_bass_kernel_spmd(nc, in_maps, core_ids=list(range(8)), trace=trace)
    out = np.zeros((B, S, D), dtype=np.float32)
    for c in range(8):
        out[c // HG] += res.results[c]["y"].astype(np.float32)
    return out, res


def kernel(x, W_qkv, W_proj):
    out, _ = run(x, W_qkv, W_proj, trace=False)
    return out
